# revision 2
# baseline (speedup 1.0000x reference)
"""Trainium2 Bass kernel for nn_AELoss (MSE + smooth loss), 8-core data-parallel.

Strategy
--------
Shard batch dim (2048) across 8 cores -> 256 rows/core. Per core, 6 steps of
(b-group of 128, c); each step DMA-loads x,y tiles [128, t-chunk, 25] with
SWDGE f32->bf16 cast (HBM reads stay f32; all on-chip compute runs in bf16,
so DVE tensor_tensor hits its 2x perf mode).

Math: working in sum/difference space kills most of the work. With
d = x - y and p = x^2 - y^2 = (x+y)(x-y):
    s_in - s_tgt per (b,c,j) = sum_t d - sum_t p + p[0] - d[T-1]
    total[b,c] = sum_{j<J-1} |s_in - s_tgt|;  smooth = mean sqrt(total)/(J*T)
    mse = mean d^2

Engine split (v2): the DVE was the critical path (5 full tensor_tensor
passes/step ~21.5us vs DMA ~18.7us). The s = x+y pass moves to the idle
TensorE: two accumulating identity matmuls per 20-row PSUM bank produce
s in f32 PSUM, and the Scalar engine (ACT, 1x rate but half idle) copies
each 4-bank region back to SBUF as bf16. DVE keeps d = x - y, p = s*d,
and the combined binary fold of (p, d) -> [128, 2, 25]. ACT also squares
d with accum_out for the per-partition MSE partial. GpSimd only issues
cast-DMAs -- real GpSimd compute poisons DVE via the shared SBUF port.
Per-core partials are returned as a [128, 20] tensor; the host combines.
"""

import os
import sys

for _p in ("/opt/trn_rl_repo", "/root/.axon_site"):
    if os.path.isdir(_p) and _p not in sys.path:
        sys.path.insert(0, _p)

import numpy as np

# bass_utils imports antenv.axon_hooks when tracing is requested (e.g. via a
# BASS_TRACE env var); the module is missing in this image, so register a
# benign stub unless someone already provided a real one.
try:
    import antenv.axon_hooks  # noqa: F401
except ImportError:
    import types

    import antenv

    _hooks = types.ModuleType("antenv.axon_hooks")
    _hook_box = [None]
    _hooks.set_axon_ntff_profile_hook = lambda h: _hook_box.__setitem__(0, h)
    _hooks.get_axon_ntff_profile_hook = lambda: _hook_box[0]
    sys.modules["antenv.axon_hooks"] = _hooks
    antenv.axon_hooks = _hooks

import concourse.bass as bass
import concourse.tile as tile
from concourse import bacc, bass_isa, masks, mybir
from concourse.bass_utils import run_bass_kernel_spmd

N_CORES = 8
B, C, T, J = 2048, 3, 300, 25
B_LOC = B // N_CORES          # 256 batch rows per core
P = 128                       # SBUF partitions
NG = B_LOC // P               # 2 b-groups per core
F32 = mybir.dt.float32
BF16 = mybir.dt.bfloat16
NSTEP = NG * C                # 6 (b-group, c) steps

BANK_T = 20                   # t-rows per PSUM bank (500 f32 of 512)
REGIONS = [(0, 80), (80, 160), (160, 240), (240, 300)]  # PSUM copy regions
CHUNKS_0 = [(0, 80), (80, 160), (160, 240), (240, 300)]  # step-0 DMA chunks
CHUNKS_N = [(0, 160), (160, 300)]                         # later DMA chunks
NMSE = len(CHUNKS_0) + (NSTEP - 1) * len(CHUNKS_N)        # 14 MSE columns


def _fold_t2(nc, fs_pool, src, res, tlen=T):
    """Sum src [P, 2, tlen, 25] over the t axis -> res [P, 2, 25] f32.

    Binary fold tree in bf16: tlen = 2*n0 + rest, halve down to 2 rows,
    final add writes f32. Supports tlen=300 (n0=128) and tlen=150 (n0=64).
    """
    v = nc.vector
    n0 = 128 if tlen >= 256 else 64
    rest = tlen - 2 * n0
    fs = fs_pool.tile([P, 2, 128, J], BF16, tag="fold_bf")
    v.tensor_add(fs[:, :, 0:n0, :], src[:, :, 0:n0, :], src[:, :, n0 : 2 * n0, :])
    v.tensor_add(fs[:, :, 0:rest, :], fs[:, :, 0:rest, :], src[:, :, 2 * n0 : tlen, :])
    n = n0 // 2
    while n >= 2:
        v.tensor_add(fs[:, :, 0:n, :], fs[:, :, 0:n, :], fs[:, :, n : 2 * n, :])
        n //= 2
    v.tensor_add(res[:, :, :], fs[:, :, 0, :], fs[:, :, 1, :])


def _body(tc, nc, x_d, y_d, out_d):
    cfg = CFG

    with (
        tc.tile_pool(name="inp", bufs=cfg["xy"]) as inp_pool,
        tc.tile_pool(name="sd", bufs=cfg["sd"]) as sd_pool,
        tc.tile_pool(name="fold", bufs=cfg["fold"]) as fold_pool,
        tc.tile_pool(name="small", bufs=3) as small_pool,
        tc.tile_pool(name="persist", bufs=1) as persist,
        tc.tile_pool(name="psum", bufs=2, space="PSUM") as psum_pool,
    ):
        totals6 = persist.tile([P, NSTEP], F32)       # per-step sum_j |s_in - s_tgt|
        mse14 = persist.tile([P, NMSE], F32)          # per-chunk sum (x-y)^2
        ident = persist.tile([P, P], BF16)            # PE stationary identity
        masks.make_identity(nc, ident[:, :])

        k = 0
        mcol = 0
        for g in range(NG):
            for c in range(C):
                # x is DMA'd straight into sd[:,1]; the in-place subtract
                # turns it into d = x - y. sd[:,0] receives s = x + y from
                # the PE/ACT path, then p = s*d in place.
                sd = sd_pool.tile([P, 2, T, J], BF16, tag="sd")
                chunks = CHUNKS_0 if k == 0 else CHUNKS_N
                for (t0, t1) in chunks:
                    tc_sz = t1 - t0
                    nc.gpsimd.dma_start(
                        out=sd[:, 1, t0:t1, :],
                        in_=x_d[g * P : (g + 1) * P, c, t0:t1, :],
                    )
                    yt = inp_pool.tile([P, 160, J], BF16, tag="y")
                    nc.gpsimd.dma_start(
                        out=yt[:, 0:tc_sz, :],
                        in_=y_d[g * P : (g + 1) * P, c, t0:t1, :],
                    )
                    # TensorE: s = x + y into PSUM, per 20-row bank, for
                    # every copy-region fully inside this chunk.
                    regs = [r for r in REGIONS if r[0] >= t0 and r[1] <= t1]
                    rtiles = []
                    for (r0, r1) in regs:
                        nb = (r1 - r0) // BANK_T
                        ps = psum_pool.tile([P, 4, 512], F32, tag="ps")
                        for b in range(nb):
                            ta = r0 + b * BANK_T
                            tb = ta + BANK_T
                            nc.tensor.matmul(
                                ps[:, b, 0 : BANK_T * J],
                                ident[:, :],
                                sd[:, 1, ta:tb, :],
                                start=True,
                                stop=False,
                            )
                            nc.tensor.matmul(
                                ps[:, b, 0 : BANK_T * J],
                                ident[:, :],
                                yt[:, ta - t0 : tb - t0, :],
                                start=False,
                                stop=True,
                            )
                        rtiles.append(ps)
                    # d = x - y in place over x (after the PE consumed x)
                    nc.vector.tensor_sub(
                        sd[:, 1, t0:t1, :], sd[:, 1, t0:t1, :], yt[:, 0:tc_sz, :]
                    )
                    # MSE partial for this chunk: sum d^2 (ACT square with
                    # accumulate; junk elementwise output goes to the
                    # consumed y tile)
                    nc.scalar.activation(
                        yt[:, 0:tc_sz, :],
                        sd[:, 1, t0:t1, :],
                        mybir.ActivationFunctionType.Square,
                        accum_out=mse14[:, mcol : mcol + 1],
                    )
                    mcol += 1
                    # ACT: copy s back to SBUF (bf16) region by region, then
                    # DVE: p = s * d in place over s.
                    for (r0, r1), ps in zip(regs, rtiles):
                        nb = (r1 - r0) // BANK_T
                        nc.scalar.activation(
                            sd[:, 0, r0:r1, :],
                            ps[:, 0:nb, 0 : BANK_T * J],
                            mybir.ActivationFunctionType.Copy,
                        )
                        nc.vector.tensor_mul(
                            sd[:, 0, r0:r1, :],
                            sd[:, 0, r0:r1, :],
                            sd[:, 1, r0:r1, :],
                        )

                # one combined fold chain: res[:,0]=Pd=sum_t p, res[:,1]=Ad=sum_t d
                res = small_pool.tile([P, 2, J], F32, tag="res")
                if k == NSTEP - 1 and cfg.get("tailfold", True):
                    # last step: fold per t-half so the first half's chain
                    # overlaps the second half's DMA -> shorter tail
                    ra = small_pool.tile([P, 2, J], F32, tag="res_a")
                    _fold_t2(nc, fold_pool, sd[:, :, 0:150, :], ra, tlen=150)
                    rb = small_pool.tile([P, 2, J], F32, tag="res_b")
                    _fold_t2(nc, fold_pool, sd[:, :, 150:300, :], rb, tlen=150)
                    nc.vector.tensor_add(res[:, :, :], ra[:, :, :], rb[:, :, :])
                else:
                    _fold_t2(nc, fold_pool, sd, res)

                # D[j] = s_in - s_tgt = Ad - Pd + p[0] - d[T-1]
                D = small_pool.tile([P, J], F32, tag="D")
                nc.vector.tensor_sub(D[:, :], res[:, 1, :], res[:, 0, :])
                nc.vector.tensor_add(D[:, :], D[:, :], sd[:, 0, 0, :])
                nc.vector.tensor_sub(D[:, :], D[:, :], sd[:, 1, T - 1, :])
                nc.vector.reduce_sum(
                    totals6[:, k : k + 1],
                    D[:, 0 : J - 1],
                    axis=mybir.AxisListType.X,
                    apply_absolute_value=True,
                )

                k += 1

        # tail: ship the raw per-partition partials; sqrt + final sums happen
        # on the host (removes the Sqrt ACT_TABLE_LOAD, reduces and
        # partition_all_reduce from the kernel's critical path). Issue the
        # early-ready pieces first -- the Sync queue is in-order, so only
        # the last step's 512B totals column rides the critical path.
        nc.sync.dma_start(out=out_d[:, NSTEP:], in_=mse14[:, :])
        nc.sync.dma_start(
            out=out_d[:, 0 : NSTEP - 1], in_=totals6[:, 0 : NSTEP - 1]
        )
        nc.sync.dma_start(
            out=out_d[:, NSTEP - 1 : NSTEP], in_=totals6[:, NSTEP - 1 : NSTEP]
        )


_NC_CACHE = None
CFG = {"xy": 6, "sd": 4, "fold": 2, "tailfold": True}


def _build():
    global _NC_CACHE
    if _NC_CACHE is not None:
        return _NC_CACHE
    nc = bacc.Bacc("TRN2", target_bir_lowering=False, debug=False, num_devices=N_CORES)
    x_d = nc.dram_tensor("inputs", [B_LOC, C, T, J], F32, kind="ExternalInput")
    y_d = nc.dram_tensor("targets", [B_LOC, C, T, J], F32, kind="ExternalInput")
    out_d = nc.dram_tensor("out", [P, NSTEP + NMSE], F32, kind="ExternalOutput")
    with tile.TileContext(nc) as tc:
        _body(tc, nc, x_d.ap(), y_d.ap(), out_d.ap())
    nc.compile()
    _NC_CACHE = nc
    return nc


def _run(inputs, targets, trace=False, **kw):
    nc = _build()
    inputs = np.ascontiguousarray(inputs, dtype=np.float32)
    targets = np.ascontiguousarray(targets, dtype=np.float32)
    in_maps = [
        {
            "inputs": inputs[i * B_LOC : (i + 1) * B_LOC],
            "targets": targets[i * B_LOC : (i + 1) * B_LOC],
        }
        for i in range(N_CORES)
    ]
    res = run_bass_kernel_spmd(
        nc, in_maps, core_ids=list(range(N_CORES)), trace=trace, **kw
    )
    mse_sum = 0.0
    smooth_sum = 0.0
    for i in range(N_CORES):
        o = np.asarray(res.results[i]["out"], dtype=np.float64)  # [P, 6+NMSE]
        totals = o[:, :NSTEP]
        smooth_sum += float(np.sqrt(totals).sum()) / (J * T)
        mse_sum += float(o[:, NSTEP:].sum())
    value = 2.0 * (mse_sum / (B * C * T * J)) + 3.0 * (smooth_sum / (B * C))
    return np.array(value, dtype=np.float32), res


def kernel(inputs, targets):
    value, _ = _run(inputs, targets)
    return value
